# revision 1
# baseline (speedup 1.0000x reference)
"""Trainium2 Bass kernel for nn_ContrastiveLoss_76501957477132.

Math (see reference): with T=0.3, n=512 tracks, Q=8, M=8192, D=128,
  yf = y.reshape(nQ, D), y_idxs[k] = k % n, track_idxs[a] = a % n.
Per track i:
  num_xy[i] = sum_{a=i mod n} sum_{k=i mod n} exp(x_a.yf_k/T)
  den_xy[i] = sum_{a=i mod n} sum_k exp(x_a.yf_k/T) - num_xy[i]
  G[i]      = sum_{a=i mod n} sum_{m=i mod n} exp(x_a.x_m/T)
  num_xx[i] = (G[i] - diag_self[i]) / 2
  den_xx[i] = sum_{a=i mod n} sum_m exp(x_a.x_m/T) - G[i]
  loss = mean(-log(num/(num+den))) / Q

Track labels are (row index mod 512) and 8192 = 16*512, so all
"positive pair" selections over 512-aligned blocks are block
diagonals.  Those terms only touch O(M*D) dot products and are
computed exactly on the host in float64.

The device computes the heavy denominators: per-track sums of
exp(x@x.T/T) and exp(x@yf.T/T).  E_xx is symmetric, so only the
upper-triangle 512x512 blocks are computed: a block's ACT accum_out
rowsum covers its own rows, and a ones-vector matmul (colsum on the
tensor engine) covers the mirrored rows.  Work is cut into
[128 x 512] "units" (matmul lhsT = 128 x-rows, rhs = one 512-row
block of xT/yfT).  Unit count per core is exactly 132 = 4 residue
groups x (15 off-diag xx + 2 diag xx + 16 xy) via band pairing
(k, 15-k), so one SPMD program serves all 8 cores; per-unit
lhsT/rhs are host-gathered inputs.

Pipeline per chunk of 3 units: 3 matmuls (bf16) -> PSUM fp32
[128,1536] -> ScalarE exp(scale=1/T, accum_out=rowsums) -> bf16
scratch; off-diag chunks additionally run 3 ones-matmuls on the
scratch accumulating colsums into a persistent [1,512] PSUM bank.
Host folds rowsum/colsum partials by row residue (mod 512) -- the
"all-reduce" -- and finishes with the tiny log/mean.
"""

import numpy as np
import ml_dtypes

M, D, N_TRACKS, Q = 8192, 128, 512, 8
NQ = N_TRACKS * Q  # 4096
TEMP = 0.3
N_CORES = 8
N_BANDS = M // N_TRACKS           # 16 row/col bands of 512
GROUPS = 4                        # residue groups (s): rows 128s..128s+127 of a band
UNITS = 33                        # units per group: 15 off + 2 diag + 16 xy
OFF_UNITS = 15
CHUNK_UNITS = 3                   # units per psum chunk [128, 1536]
CHUNKS_PER_GROUP = UNITS // CHUNK_UNITS  # 11
OFF_CHUNKS = OFF_UNITS // CHUNK_UNITS    # 5
ACTS_PER_CORE = GROUPS * CHUNKS_PER_GROUP  # 44

_CACHED = {}


def _core_units(k):
    """Unit descriptors for core k: list of (band, rhs_kind, rhs_idx).

    rhs_kind: 'x' -> xT block rhs_idx, 'y' -> yfT block rhs_idx.
    Order: 15 off-diag xx (per-unit lhsT), then a 9-unit all-band-A run
    (diag + xy) and a 9-unit all-band-B run, each sharing one lhsT slot.
    Bands A=k, B=15-k.
    """
    A, B = k, (N_BANDS - 1) - k
    units = []
    units += [(A, "x", c) for c in range(A + 1, N_BANDS)]   # 15-k
    units += [(B, "x", c) for c in range(B + 1, N_BANDS)]   # k
    assert len(units) == OFF_UNITS
    units += [(A, "x", A)] + [(A, "y", q) for q in range(Q)]
    units += [(B, "x", B)] + [(B, "y", q) for q in range(Q)]
    assert len(units) == UNITS
    return units


def _build_module():
    import concourse.bacc as bacc
    import concourse.tile as tile
    import concourse.mybir as mybir

    nc = bacc.Bacc(None, target_bir_lowering=False)
    bf16 = mybir.dt.bfloat16
    f32 = mybir.dt.float32

    # Stationary operands: per-unit [128,128] slices for the 15 off-diag
    # units; the two 9-unit diag+xy runs share one lhsT slot per band.
    # rhs blocks are deduplicated: A-xy and B-xy units share yfT blocks, so
    # only 25 distinct 512-column blocks are stored (17 xT + 8 yfT).
    RHS_BLOCKS = UNITS - Q  # 25
    lhsT_off_d = nc.dram_tensor(
        "lhsT_off", [128, GROUPS, OFF_UNITS, 128], bf16, kind="ExternalInput"
    )
    lhsT_run_d = nc.dram_tensor(
        "lhsT_run", [128, GROUPS, 2, 128], bf16, kind="ExternalInput"
    )
    rhs_d = nc.dram_tensor("rhs", [128, RHS_BLOCKS, 512], bf16, kind="ExternalInput")
    acc_d = nc.dram_tensor("acc", [128, ACTS_PER_CORE], f32, kind="ExternalOutput")
    cs_d = nc.dram_tensor("cs", [1, 512], f32, kind="ExternalOutput")

    with tile.TileContext(nc) as tc:
        with (
            tc.tile_pool(name="consts", bufs=1) as consts,
            tc.tile_pool(name="accp", bufs=1) as accp,
            tc.tile_pool(name="scratch", bufs=3) as scratch_pool,
            tc.tile_pool(name="psum", bufs=2, space="PSUM") as psum_pool,
            tc.tile_pool(name="cspsum", bufs=1, space="PSUM") as cs_pool,
        ):
            # Input DMAs split into consumption-ordered pieces spread over
            # three DGE queues (SP hardware DGE, GpSimd software DGE, and
            # Scalar DGE for mid pieces while ACT is still idle) so the
            # first matmul starts early and the load hides under compute.
            rhs_splits = [(0, 1), (1, 3), (3, 6), (6, 10), (10, 15), (15, 18),
                          (18, 21), (21, RHS_BLOCKS)]
            rhs_tiles = {}
            for lo, hi in rhs_splits:
                rhs_tiles[lo] = consts.tile(
                    [128, hi - lo, 512], bf16, tag=f"rhs{lo}", name=f"rhs{lo}"
                )

            off_splits = {0: [(0, 3), (3, 9), (9, OFF_UNITS)]}
            for s in range(1, GROUPS):
                off_splits[s] = [(0, OFF_UNITS)]
            off_tiles = {}
            for s in range(GROUPS):
                for lo, hi in off_splits[s]:
                    off_tiles[(s, lo)] = consts.tile(
                        [128, hi - lo, 128],
                        bf16,
                        tag=f"lhsToff{s}_{lo}",
                        name=f"lhsToff{s}_{lo}",
                    )
            run_tile = consts.tile(
                [128, GROUPS, 2, 128], bf16, tag="lhsTrun", name="lhsTrun"
            )

            def rhs_dma(eng, lo):
                hi = dict(rhs_splits)[lo]
                eng.dma_start(rhs_tiles[lo][:], rhs_d[:, lo:hi, :])

            def off_dma(eng, s, lo):
                hi = dict(off_splits[s])[lo]
                eng.dma_start(off_tiles[(s, lo)][:], lhsT_off_d[:, s, lo:hi, :])

            # All input loads go through ONE queue in strict consumption
            # order: the DMA engines fair-share bandwidth across queues, so
            # spreading the loads lets late bulk pieces starve the pieces the
            # first chunks need.  Serial order paces arrivals with compute.
            # Run-chunks execute first per group (tiny shared lhsT), so the
            # big off-diag lhsT gathers hide under their compute.
            nc.sync.dma_start(run_tile[:, 0], lhsT_run_d[:, 0])
            rhs_dma(nc.sync, 15)
            rhs_dma(nc.sync, 18)
            rhs_dma(nc.sync, 21)
            nc.sync.dma_start(run_tile[:, 1:], lhsT_run_d[:, 1:])
            off_dma(nc.sync, 0, 0)
            rhs_dma(nc.sync, 0)
            rhs_dma(nc.sync, 1)
            rhs_dma(nc.sync, 3)
            off_dma(nc.sync, 0, 3)
            rhs_dma(nc.sync, 6)
            off_dma(nc.sync, 0, 9)
            rhs_dma(nc.sync, 10)
            for s in range(1, GROUPS):
                off_dma(nc.sync, s, 0)

            def rhs_ap(u):
                blk = u if u < 25 else u - 9  # B-xy units reuse yfT blocks
                for lo, hi in rhs_splits:
                    if lo <= blk < hi:
                        return rhs_tiles[lo][:, blk - lo, :]
                raise AssertionError

            def lhsT_ap(s, u):
                if u < OFF_UNITS:
                    for lo, hi in off_splits[s]:
                        if lo <= u < hi:
                            return off_tiles[(s, lo)][:, u - lo, :]
                    raise AssertionError
                return run_tile[:, s, 0 if u < OFF_UNITS + 9 else 1, :]

            ones_sb = consts.tile([128, 1], bf16, tag="ones")
            nc.vector.memset(ones_sb[:], 1.0)

            acc_sb = accp.tile([128, ACTS_PER_CORE], f32)
            cs_ps = cs_pool.tile([1, 512], f32, name="cs_ps")

            n_ones = GROUPS * OFF_CHUNKS * CHUNK_UNITS  # 60
            ones_done = 0
            pending = []  # delayed ones-matmuls: (scratch_tile, slice_idx)

            def flush_pending():
                nonlocal ones_done
                for sc, j in pending:
                    ones_done += 1
                    nc.tensor.matmul(
                        cs_ps[:],
                        ones_sb[:],
                        sc[:, j * 512 : (j + 1) * 512],
                        start=(ones_done == 1),
                        stop=(ones_done == n_ones),
                        skip_group_check=True,
                    )
                pending.clear()

            cs_written = False
            chunk_order = list(range(OFF_CHUNKS, CHUNKS_PER_GROUP)) + list(
                range(OFF_CHUNKS)
            )  # run-chunks first, then off-chunks
            for s in range(GROUPS):
                for j in chunk_order:
                    ps = psum_pool.tile([128, CHUNK_UNITS * 512], f32)
                    for e in range(CHUNK_UNITS):
                        u = j * CHUNK_UNITS + e
                        nc.tensor.matmul(
                            ps[:, e * 512 : (e + 1) * 512],
                            lhsT_ap(s, u),
                            rhs_ap(u),
                            start=True,
                            stop=True,
                        )
                    # ones-matmuls for the previous off-chunk run after this
                    # chunk's matmuls so the PE never waits on the ACT
                    flush_pending()
                    if ones_done == n_ones and not cs_written:
                        # colsums complete well before the last chunks: drain
                        # them now so the writeback hides under compute
                        cs_written = True
                        cs_sb = accp.tile([1, 512], f32, tag="cs_sb")
                        nc.vector.tensor_copy(cs_sb[:], cs_ps[:])
                        nc.sync.dma_start(cs_d[:], cs_sb[:])
                    slot = s * CHUNKS_PER_GROUP + j
                    sc = scratch_pool.tile([128, CHUNK_UNITS * 512], bf16)
                    nc.scalar.activation(
                        out=sc[:],
                        in_=ps[:],
                        func=mybir.ActivationFunctionType.Exp,
                        scale=1.0 / TEMP,
                        accum_out=acc_sb[:, slot : slot + 1],
                    )
                    if j < OFF_CHUNKS:
                        pending.extend((sc, e) for e in range(CHUNK_UNITS))
                # this group's rowsum accumulators are final: ship them
                lo, hi = s * CHUNKS_PER_GROUP, (s + 1) * CHUNKS_PER_GROUP
                nc.sync.dma_start(acc_d[:, lo:hi], acc_sb[:, lo:hi])
            flush_pending()
            if not cs_written:
                cs_sb = accp.tile([1, 512], f32, tag="cs_sb")
                nc.vector.tensor_copy(cs_sb[:], cs_ps[:])
                nc.sync.dma_start(cs_d[:], cs_sb[:])
    nc.compile()
    return nc


def _get_module():
    if "nc" not in _CACHED:
        _CACHED["nc"] = _build_module()
    return _CACHED["nc"]


def _positive_terms(x64, yf64):
    """num_xy, G_diag, diag_self as float64 [512] vectors (exact math)."""
    xs = x64.reshape(N_BANDS, N_TRACKS, D)              # [16, 512, 128]
    yfs = yf64.reshape(NQ // N_TRACKS, N_TRACKS, D)     # [8, 512, 128]
    dxx = np.einsum("rid,cid->rci", xs, xs)             # [16, 16, 512]
    dxy = np.einsum("rid,qid->rqi", xs, yfs)            # [16, 8, 512]
    G = np.exp(dxx / TEMP).sum(axis=(0, 1))             # [512]
    diag_self = np.exp(np.einsum("rid,rid->ri", xs, xs) / TEMP).sum(axis=0)
    num_xy = np.exp(dxy / TEMP).sum(axis=(0, 1))        # [512]
    return num_xy, G, diag_self


def _finish(rs_seg, num_xy, G, diag_self):
    num = num_xy + (G - diag_self) / 2.0
    den = rs_seg - num_xy - G
    loss = np.mean(-np.log(num / (den + num))) / Q
    return np.asarray(loss, dtype=np.float32)


def _numpy_fallback(x, track_idxs, y):
    """Exact general-track reference in numpy (safety net only)."""
    x64 = x.astype(np.float64)
    yf64 = y.reshape(NQ, D).astype(np.float64)
    t = track_idxs.astype(np.int64)
    y_idxs = np.tile(np.arange(N_TRACKS, dtype=np.int64), Q)
    E_xy = np.exp(x64 @ yf64.T / TEMP)
    Sx = np.zeros((N_TRACKS, NQ))
    np.add.at(Sx, t, E_xy)
    Py = (y_idxs[:, None] == np.arange(N_TRACKS)[None, :]).astype(np.float64)
    num_xy = np.einsum("ik,ki->i", Sx, Py)
    den_xy = Sx.sum(axis=1) - num_xy
    E_xx = np.exp(x64 @ x64.T / TEMP)
    Sxx = np.zeros((N_TRACKS, M))
    np.add.at(Sxx, t, E_xx)
    Px = (t[:, None] == np.arange(N_TRACKS)[None, :]).astype(np.float64)
    G_diag = np.einsum("im,mi->i", Sxx, Px)
    diag_self = np.zeros(N_TRACKS)
    np.add.at(diag_self, t, np.diagonal(E_xx))
    num_xx = (G_diag - diag_self) / 2.0
    den_xx = Sxx.sum(axis=1) - G_diag
    num = num_xy + num_xx
    den = den_xy + den_xx
    loss = np.mean(-np.log(num / (den + num))) / Q
    return np.asarray(loss, dtype=np.float32)


def kernel(x, track_idxs, y):
    x = np.asarray(x, dtype=np.float32)
    y = np.asarray(y, dtype=np.float32)
    track_idxs = np.asarray(track_idxs)

    expected_tracks = np.arange(M, dtype=np.int64) % N_TRACKS
    if (
        x.shape != (M, D)
        or y.shape != (N_TRACKS, Q, D)
        or not np.array_equal(track_idxs.astype(np.int64), expected_tracks)
    ):
        return _numpy_fallback(x, track_idxs, y)

    from concourse.bass_utils import run_bass_kernel_spmd

    yf = np.ascontiguousarray(y.reshape(NQ, D))
    xT = np.ascontiguousarray(x.T).astype(ml_dtypes.bfloat16)    # [128, 8192]
    yfT = np.ascontiguousarray(yf.T).astype(ml_dtypes.bfloat16)  # [128, 4096]
    xT_blocks = xT.reshape(128, N_BANDS, 512)
    yfT_blocks = yfT.reshape(128, Q, 512)

    in_maps = []
    for k in range(N_CORES):
        units = _core_units(k)
        rhs = np.stack(
            [
                (xT_blocks[:, idx] if kind == "x" else yfT_blocks[:, idx])
                for (_band, kind, idx) in units[: UNITS - Q]
            ],
            axis=1,
        )  # [128, 25, 512]: 17 xT blocks + 8 yfT blocks (shared A/B xy)

        def subtile(band, s):
            t = 4 * band + s
            return xT[:, 128 * t : 128 * (t + 1)]

        lhsT_off = np.empty((128, GROUPS, OFF_UNITS, 128), dtype=ml_dtypes.bfloat16)
        lhsT_run = np.empty((128, GROUPS, 2, 128), dtype=ml_dtypes.bfloat16)
        for s in range(GROUPS):
            for u in range(OFF_UNITS):
                lhsT_off[:, s, u, :] = subtile(units[u][0], s)
            lhsT_run[:, s, 0, :] = subtile(k, s)
            lhsT_run[:, s, 1, :] = subtile((N_BANDS - 1) - k, s)
        in_maps.append(
            {
                "lhsT_off": np.ascontiguousarray(lhsT_off),
                "lhsT_run": np.ascontiguousarray(lhsT_run),
                "rhs": np.ascontiguousarray(rhs),
            }
        )

    nc = _get_module()
    res = run_bass_kernel_spmd(nc, in_maps, core_ids=list(range(N_CORES)))
    _CACHED["last_res"] = res

    # Fold partial sums by row residue (mod 512): group s covers residues
    # 128s + p; colsums fold by in-block column position directly.
    rs_seg = np.zeros(N_TRACKS, dtype=np.float64)
    for k in range(N_CORES):
        acc = np.asarray(res.results[k]["acc"], dtype=np.float64)  # [128, 44]
        per_group = acc.reshape(128, GROUPS, CHUNKS_PER_GROUP).sum(axis=2)
        rs_seg += per_group.T.reshape(N_TRACKS)  # i = 128*s + p
        rs_seg += np.asarray(res.results[k]["cs"], dtype=np.float64).reshape(-1)

    num_xy, G, diag_self = _positive_terms(
        x.astype(np.float64), yf.astype(np.float64)
    )
    return _finish(rs_seg, num_xy, G, diag_self)



# revision 7
# speedup vs baseline: 1.0554x; 1.0554x over previous
"""Trainium2 Bass kernel for nn_ContrastiveLoss_76501957477132.

Math (see reference): with T=0.3, n=512 tracks, Q=8, M=8192, D=128,
  yf = y.reshape(nQ, D), y_idxs[k] = k % n, track_idxs[a] = a % n.
Per track i:
  num_xy[i] = sum_{a=i mod n} sum_{k=i mod n} exp(x_a.yf_k/T)
  den_xy[i] = sum_{a=i mod n} sum_k exp(x_a.yf_k/T) - num_xy[i]
  G[i]      = sum_{a=i mod n} sum_{m=i mod n} exp(x_a.x_m/T)
  num_xx[i] = (G[i] - diag_self[i]) / 2
  den_xx[i] = sum_{a=i mod n} sum_m exp(x_a.x_m/T) - G[i]
  loss = mean(-log(num/(num+den))) / Q

Track labels are (row index mod 512), so the device only needs
residue-class sums of exp over the E_xx (symmetric; upper triangle
only, colsums complete the mirrored rows) and E_xy matrices.
Positive-pair terms (tiny) are exact on the host in float64.

Work per core: 4 residue groups (s) x 33 [128x512] matmul units via
band pairing (A=k, B=15-k): 18 run units (2 diag + 16 xy) + 15 off.

v3 pipeline -- the kernel is ACT(exp)-bound, 56.3us of exp columns
is the floor, so ScalarE does *nothing but* 36 big exp instructions:
- PSUM 2 x [128,2048] fp32 (all 8 banks); chunks of <=4 units.
- Chunks per group: 4xr4, r2 (run), 3xo4, o3 (off).  The r2 chunk
  runs first (fastest start); each group ends with the o3 chunk
  (1536 cols covers the next group's PE refill, no ACT bubble).
- Rowsums on VectorE in two stages (2x perf mode needs 16-bit dtypes
  end to end): per run chunk a [128,cols]->[128,cols/128] bf16
  partial reduce; per group the off-diag exp is accumulated into a
  per-group [128,2048] bf16 tile (tensor_copy init, then in-place
  adds -- all 2x/4x mode) which is partial-reduced once.  A single
  fp32 stage-2 reduce at the end folds all partials into [128,24].
- Colsums: the 4 per-group accumulator tiles ship to DRAM (positions
  fold mod 512 on the host); group 3 runs its off chunks first so
  the last writeback hides under its run chunks.
- ACT spline tables are warmed by a dummy exp at t=0.  No gpsimd
  anywhere (its DGE drain lengthens the end-of-kernel barrier).
"""

import numpy as np
import ml_dtypes

M, D, N_TRACKS, Q = 8192, 128, 512, 8
NQ = N_TRACKS * Q  # 4096
TEMP = 0.3
N_CORES = 8
N_BANDS = M // N_TRACKS           # 16 row/col bands of 512
GROUPS = 4                        # residue groups (s): rows 128s..128s+127 of a band
UNITS = 33                        # units per group: 18 run (2 diag + 16 xy) + 15 off
RUN_UNITS = 18
OFF_UNITS = 15
CHUNKS = 9                        # c0..c3 r4, c4 r2, c5..c7 o4, c8 o3
RS_SLOTS = 6                      # rowsum slots per group: c0..c4 + off total

_CACHED = {}


def _unit_aps(u):
    """(lhsT_kind, lhsT_idx, rhs_slot) for unit u of a group.

    lhsT_kind 0 -> lhsT_run[:, s, idx]; 1 -> lhsT_off[:, s, idx].
    Unit order: diag A, A-xy q0..7, diag B, B-xy q0..7, off 0..14.
    rhs slots: 0..14 off blocks, 15 diag A, 16..23 yfT, 24 diag B.
    """
    if u == 0:
        return (0, 0, 15)
    if 1 <= u <= 8:
        return (0, 0, 15 + u)
    if u == 9:
        return (0, 1, 24)
    if 10 <= u <= 17:
        return (0, 1, 6 + u)
    return (1, u - RUN_UNITS, u - RUN_UNITS)


def _chunk_units(j):
    if j == 4:
        return [16, 17]
    if j == 8:
        return [30, 31, 32]
    return list(range(4 * j, 4 * j + 4)) if j < 4 else list(range(4 * j - 2, 4 * j + 2))


def _build_module():
    import concourse.bacc as bacc
    import concourse.tile as tile
    import concourse.mybir as mybir

    nc = bacc.Bacc(None, target_bir_lowering=False)
    bf16 = mybir.dt.bfloat16
    f32 = mybir.dt.float32
    ADD = mybir.AluOpType.add
    AXX = mybir.AxisListType.X

    RHS_BLOCKS = 25  # 15 off + diag A + 8 yfT (shared A/B xy) + diag B
    lhsT_off_d = nc.dram_tensor(
        "lhsT_off", [128, GROUPS, OFF_UNITS, 128], bf16, kind="ExternalInput"
    )
    lhsT_run_d = nc.dram_tensor(
        "lhsT_run", [128, GROUPS, 2, 128], bf16, kind="ExternalInput"
    )
    rhs_d = nc.dram_tensor("rhs", [128, RHS_BLOCKS, 512], bf16, kind="ExternalInput")
    acc_d = nc.dram_tensor("acc", [128, GROUPS * RS_SLOTS], f32, kind="ExternalOutput")
    cs_d = nc.dram_tensor("cs", [128, GROUPS, 2048], bf16, kind="ExternalOutput")

    with tile.TileContext(nc) as tc:
        with (
            tc.tile_pool(name="consts", bufs=1) as consts,
            tc.tile_pool(name="accp", bufs=1) as accp,
            tc.tile_pool(name="scratch", bufs=3) as scratch_pool,
            tc.tile_pool(name="psum", bufs=2, space="PSUM") as psum_pool,
        ):
            # --- input tiles -------------------------------------------------
            rhs_splits = [(22, 24), (15, 19), (19, 22), (24, 25),
                          (0, 4), (4, 8), (8, 15)]
            rhs_tiles = {}
            for lo, hi in rhs_splits:
                rhs_tiles[lo] = consts.tile(
                    [128, hi - lo, 512], bf16, tag=f"rhs{lo}", name=f"rhs{lo}"
                )
            off_tiles = {}
            for s in range(GROUPS):
                off_tiles[s] = consts.tile(
                    [128, OFF_UNITS, 128], bf16,
                    tag=f"lhsToff{s}", name=f"lhsToff{s}",
                )
            run_tile = consts.tile(
                [128, GROUPS, 2, 128], bf16, tag="lhsTrun", name="lhsTrun"
            )

            # ACT table warmup: dummy exp on a const tile, no DMA deps.
            ones_sb = consts.tile([128, 1], bf16, tag="ones")
            dummy_sb = consts.tile([128, 1], f32, tag="dummy")
            nc.vector.memset(ones_sb[:], 1.0)
            nc.scalar.activation(
                out=dummy_sb[:], in_=ones_sb[:],
                func=mybir.ActivationFunctionType.Exp,
            )

            # Input loads in strict consumption order on the sync queue.
            def rhs_dma(lo):
                hi = dict(rhs_splits)[lo]
                nc.sync.dma_start(rhs_tiles[lo][:], rhs_d[:, lo:hi, :])

            nc.sync.dma_start(run_tile[:, 0], lhsT_run_d[:, 0])
            for lo, _hi in rhs_splits[:4]:
                rhs_dma(lo)
            nc.sync.dma_start(off_tiles[0][:], lhsT_off_d[:, 0])
            for lo, _hi in rhs_splits[4:]:
                rhs_dma(lo)
            nc.sync.dma_start(run_tile[:, 1:], lhsT_run_d[:, 1:])
            for s in range(1, GROUPS):
                nc.sync.dma_start(off_tiles[s][:], lhsT_off_d[:, s])

            def rhs_ap(slot):
                for lo, hi in rhs_splits:
                    if lo <= slot < hi:
                        return rhs_tiles[lo][:, slot - lo, :]
                raise AssertionError

            def lhsT_ap(s, u):
                kind, idx, _slot = _unit_aps(u)
                if kind == 0:
                    return run_tile[:, s, idx, :]
                return off_tiles[s][:, idx, :]

            # --- accumulators ------------------------------------------------
            # rowsum partials: [128, 24 slots, 16] bf16; slot s*6+j for run
            # chunks j=c4,c0..c3 -> 0..4, off total -> 5.
            rp_sb = accp.tile([128, GROUPS * RS_SLOTS, 16], bf16, tag="rp")
            acc_sb = accp.tile([128, GROUPS * RS_SLOTS], f32, tag="acc")
            cs_tiles = [
                accp.tile([128, 16, 128], bf16, tag=f"cs{s}", name=f"cs{s}")
                for s in range(GROUPS)
            ]
            nc.vector.memset(rp_sb[:], 0.0)

            def do_chunk(s, j, first_off):
                units = _chunk_units(j)
                cols = 512 * len(units)
                n16 = cols // 128
                ps = psum_pool.tile([128, 2048], f32)
                for e, u in enumerate(units):
                    _kind, _idx, slot = _unit_aps(u)
                    nc.tensor.matmul(
                        ps[:, e * 512:(e + 1) * 512],
                        lhsT_ap(s, u),
                        rhs_ap(slot),
                        start=True,
                        stop=True,
                    )
                sc = scratch_pool.tile([128, 16, 128], bf16)
                nc.scalar.activation(
                    out=sc[:, :n16, :], in_=ps[:, :cols],
                    func=mybir.ActivationFunctionType.Exp,
                    scale=1.0 / TEMP,
                )
                cs = cs_tiles[s]
                if j >= 5:  # off chunk: accumulate exp values for colsums
                    if first_off:
                        nc.vector.tensor_copy(cs[:, :n16, :], sc[:, :n16, :])
                    else:
                        nc.vector.tensor_tensor(
                            cs[:, :n16, :], sc[:, :n16, :], cs[:, :n16, :], ADD
                        )
                else:       # run chunk: stage-1 partial rowsum (2x mode)
                    slot_i = s * RS_SLOTS + (0 if j == 4 else 1 + j)
                    with nc.allow_low_precision(reason="bf16 rowsum partials"):
                        nc.vector.tensor_reduce(
                            rp_sb[:, slot_i, :n16], sc[:, :n16, :], AXX, ADD,
                        )

            for s in range(GROUPS):
                order = ([4, 0, 1, 2, 3, 5, 6, 7, 8] if s < GROUPS - 1
                         else [5, 6, 7, 8, 4, 0, 1, 2, 3])
                seen_off = False
                for j in order:
                    do_chunk(s, j, first_off=(j >= 5 and not seen_off))
                    seen_off = seen_off or j >= 5
                    if j == 8:
                        # group's off accumulation complete: stage-1 reduce
                        # its rowsums and ship the colsum tile
                        with nc.allow_low_precision(reason="bf16 partials"):
                            nc.vector.tensor_reduce(
                                rp_sb[:, s * RS_SLOTS + 5, :],
                                cs_tiles[s][:], AXX, ADD,
                            )
                        nc.sync.dma_start(cs_d[:, s], cs_tiles[s][:])

            # stage-2: fold all bf16 partials to fp32 rowsums, ship
            nc.vector.tensor_reduce(acc_sb[:], rp_sb[:], AXX, ADD)
            nc.sync.dma_start(acc_d[:], acc_sb[:])
    nc.compile()
    return nc


def _get_module():
    if "nc" not in _CACHED:
        _CACHED["nc"] = _build_module()
    return _CACHED["nc"]


def _core_off_blocks(k):
    """Off-diag (band, col) pairs for core k, in unit order 0..14."""
    A, B = k, (N_BANDS - 1) - k
    pairs = [(A, c) for c in range(A + 1, N_BANDS)]
    pairs += [(B, c) for c in range(B + 1, N_BANDS)]
    assert len(pairs) == OFF_UNITS
    return pairs


def _positive_terms(x64, yf64):
    """num_xy, G_diag, diag_self as float64 [512] vectors (exact math)."""
    xs = x64.reshape(N_BANDS, N_TRACKS, D)              # [16, 512, 128]
    yfs = yf64.reshape(NQ // N_TRACKS, N_TRACKS, D)     # [8, 512, 128]
    dxx = np.einsum("rid,cid->rci", xs, xs)             # [16, 16, 512]
    dxy = np.einsum("rid,qid->rqi", xs, yfs)            # [16, 8, 512]
    G = np.exp(dxx / TEMP).sum(axis=(0, 1))             # [512]
    diag_self = np.exp(np.einsum("rid,rid->ri", xs, xs) / TEMP).sum(axis=0)
    num_xy = np.exp(dxy / TEMP).sum(axis=(0, 1))        # [512]
    return num_xy, G, diag_self


def _finish(rs_seg, num_xy, G, diag_self):
    num = num_xy + (G - diag_self) / 2.0
    den = rs_seg - num_xy - G
    loss = np.mean(-np.log(num / (den + num))) / Q
    return np.asarray(loss, dtype=np.float32)


def _numpy_fallback(x, track_idxs, y):
    """Exact general-track reference in numpy (safety net only)."""
    x64 = x.astype(np.float64)
    yf64 = y.reshape(NQ, D).astype(np.float64)
    t = track_idxs.astype(np.int64)
    y_idxs = np.tile(np.arange(N_TRACKS, dtype=np.int64), Q)
    E_xy = np.exp(x64 @ yf64.T / TEMP)
    Sx = np.zeros((N_TRACKS, NQ))
    np.add.at(Sx, t, E_xy)
    Py = (y_idxs[:, None] == np.arange(N_TRACKS)[None, :]).astype(np.float64)
    num_xy = np.einsum("ik,ki->i", Sx, Py)
    den_xy = Sx.sum(axis=1) - num_xy
    E_xx = np.exp(x64 @ x64.T / TEMP)
    Sxx = np.zeros((N_TRACKS, M))
    np.add.at(Sxx, t, E_xx)
    Px = (t[:, None] == np.arange(N_TRACKS)[None, :]).astype(np.float64)
    G_diag = np.einsum("im,mi->i", Sxx, Px)
    diag_self = np.zeros(N_TRACKS)
    np.add.at(diag_self, t, np.diagonal(E_xx))
    num_xx = (G_diag - diag_self) / 2.0
    den_xx = Sxx.sum(axis=1) - G_diag
    num = num_xy + num_xx
    den = den_xy + den_xx
    loss = np.mean(-np.log(num / (den + num))) / Q
    return np.asarray(loss, dtype=np.float32)


def kernel(x, track_idxs, y):
    x = np.asarray(x, dtype=np.float32)
    y = np.asarray(y, dtype=np.float32)
    track_idxs = np.asarray(track_idxs)

    expected_tracks = np.arange(M, dtype=np.int64) % N_TRACKS
    if (
        x.shape != (M, D)
        or y.shape != (N_TRACKS, Q, D)
        or not np.array_equal(track_idxs.astype(np.int64), expected_tracks)
    ):
        return _numpy_fallback(x, track_idxs, y)

    from concourse.bass_utils import run_bass_kernel_spmd

    yf = np.ascontiguousarray(y.reshape(NQ, D))
    xT = np.ascontiguousarray(x.T).astype(ml_dtypes.bfloat16)    # [128, 8192]
    yfT = np.ascontiguousarray(yf.T).astype(ml_dtypes.bfloat16)  # [128, 4096]
    xT_blocks = xT.reshape(128, N_BANDS, 512)
    yfT_blocks = yfT.reshape(128, Q, 512)

    in_maps = []
    for k in range(N_CORES):
        A, B = k, (N_BANDS - 1) - k
        pairs = _core_off_blocks(k)
        rhs = np.empty((128, 25, 512), dtype=ml_dtypes.bfloat16)
        for u, (_band, c) in enumerate(pairs):
            rhs[:, u] = xT_blocks[:, c]
        rhs[:, 15] = xT_blocks[:, A]
        rhs[:, 16:24] = yfT_blocks
        rhs[:, 24] = xT_blocks[:, B]

        def subtile(band, s):
            t = 4 * band + s
            return xT[:, 128 * t: 128 * (t + 1)]

        lhsT_off = np.empty((128, GROUPS, OFF_UNITS, 128), dtype=ml_dtypes.bfloat16)
        lhsT_run = np.empty((128, GROUPS, 2, 128), dtype=ml_dtypes.bfloat16)
        for s in range(GROUPS):
            for u, (band, _c) in enumerate(pairs):
                lhsT_off[:, s, u, :] = subtile(band, s)
            lhsT_run[:, s, 0, :] = subtile(A, s)
            lhsT_run[:, s, 1, :] = subtile(B, s)
        in_maps.append(
            {
                "lhsT_off": np.ascontiguousarray(lhsT_off),
                "lhsT_run": np.ascontiguousarray(lhsT_run),
                "rhs": np.ascontiguousarray(rhs),
            }
        )

    nc = _get_module()
    res = run_bass_kernel_spmd(nc, in_maps, core_ids=list(range(N_CORES)))
    _CACHED["last_res"] = res

    # Fold rowsum partials by row residue (128s + p) and colsum partials by
    # in-block column position (mod 512) -- the "all-reduce" -- on the host.
    rs_seg = np.zeros(N_TRACKS, dtype=np.float64)
    for k in range(N_CORES):
        acc = np.asarray(res.results[k]["acc"], dtype=np.float64)  # [128, 24]
        per_group = acc.reshape(128, GROUPS, RS_SLOTS).sum(axis=2)
        rs_seg += per_group.T.reshape(N_TRACKS)  # i = 128*s + p
        # cs rowsum part was already folded into acc via the per-group
        # reduce; cs tiles supply only the colsum (mirror) part here.
        cs = np.asarray(res.results[k]["cs"], dtype=np.float64)    # [128, 4, 2048]
        rs_seg += cs.reshape(128 * GROUPS, 4, 512).sum(axis=(0, 1))

    num_xy, G, diag_self = _positive_terms(
        x.astype(np.float64), yf.astype(np.float64)
    )
    return _finish(rs_seg, num_xy, G, diag_self)


# revision 24
# speedup vs baseline: 1.1018x; 1.0440x over previous
"""Trainium2 Bass kernel for nn_ContrastiveLoss_76501957477132.

Math (see reference): with T=0.3, n=512 tracks, Q=8, M=8192, D=128,
  yf = y.reshape(nQ, D), y_idxs[k] = k % n, track_idxs[a] = a % n.
Per track i:
  num_xy[i] = sum_{a=i mod n} sum_{k=i mod n} exp(x_a.yf_k/T)
  den_xy[i] = sum_{a=i mod n} sum_k exp(x_a.yf_k/T) - num_xy[i]
  G[i]      = sum_{a=i mod n} sum_{m=i mod n} exp(x_a.x_m/T)
  num_xx[i] = (G[i] - diag_self[i]) / 2
  den_xx[i] = sum_{a=i mod n} sum_m exp(x_a.x_m/T) - G[i]
  loss = mean(-log(num/(num+den))) / Q

Track labels are (row index mod 512), so the device only needs
residue-class sums of exp over the E_xx (symmetric; upper triangle
only, colsums complete the mirrored rows) and E_xy matrices.
Positive-pair terms (tiny) are exact on the host in float64.

Work per core: 4 residue groups (s) x 33 [128x512] matmul units via
band pairing (A=k, B=15-k): 18 run units (2 diag + 16 xy) + 15 off.

v4 pipeline -- the kernel is ACT(exp)-bound, 56.3us of exp columns
is the floor, so ScalarE does *nothing but* 36 big exp instructions:
- PSUM 2 x [128,2048] fp32 (all 8 banks); chunks of <=4 units.
- Chunks per group: 4xr4, r2 (run), 3xo4, o3 (off).  The r2 chunk
  runs first (fastest start); each group ends with the o3 chunk
  (1536 cols covers the next group's PE refill, no ACT bubble).
- VectorE tensor_reduce has no 2x uop variant (measured ~2.2us per
  [128,2048] regardless of dtype), so the device reduces NOTHING:
  every chunk's exp scratch is compressed into per-group bf16
  accumulator tiles with tensor_copy / in-place tensor_tensor adds
  only (4x / 2x perf modes, ~0.6-1.2us per chunk), run and off
  chunks into separate tiles.  All 8 tiles ship to DRAM and the
  host folds rowsums (run+off tiles) and mirror colsums (off tiles,
  positions mod 512) in float64.
- Group 3 runs its off chunks first and its r2 chunk last so both
  final writebacks hide under compute.
- ACT spline tables are warmed by a dummy exp at t=0.  No gpsimd
  anywhere (its DGE drain lengthens the end-of-kernel barrier).
"""

import numpy as np
import ml_dtypes

M, D, N_TRACKS, Q = 8192, 128, 512, 8
NQ = N_TRACKS * Q  # 4096
TEMP = 0.3
N_CORES = 8
N_BANDS = M // N_TRACKS           # 16 row/col bands of 512
GROUPS = 4                        # residue groups (s): rows 128s..128s+127 of a band
UNITS = 33                        # units per group: 18 run (2 diag + 16 xy) + 15 off
RUN_UNITS = 18
OFF_UNITS = 15
CHUNKS = 9                        # c0..c3 r4, c4 r2, c5..c7 o4, c8 o3

_CACHED = {}


def _unit_aps(u):
    """(lhsT_kind, lhsT_idx, rhs_slot) for unit u of a group.

    lhsT_kind 0 -> lhsT_run[:, s, idx]; 1 -> lhsT_off[:, s, idx].
    Unit order: diag A, A-xy q0..7, diag B, B-xy q0..7, off 0..14.
    rhs slots: 0..14 off blocks, 15 diag A, 16..23 yfT, 24 diag B.
    """
    if u == 0:
        return (0, 0, 15)
    if 1 <= u <= 8:
        return (0, 0, 15 + u)
    if u == 9:
        return (0, 1, 24)
    if 10 <= u <= 17:
        return (0, 1, 6 + u)
    return (1, u - RUN_UNITS, u - RUN_UNITS)


def _chunk_units(s, j):
    """Units of chunk j in group s (c4 = 2-unit starter, c8 = 3-unit)."""
    del s
    if j == 4:
        return [16, 17]
    if j == 8:
        return [30, 31, 32]
    return list(range(4 * j, 4 * j + 4)) if j < 4 else list(range(4 * j - 2, 4 * j + 2))


def _build_module():
    import concourse.bacc as bacc
    import concourse.tile as tile
    import concourse.mybir as mybir

    nc = bacc.Bacc(None, target_bir_lowering=False)
    bf16 = mybir.dt.bfloat16
    f32 = mybir.dt.float32
    ADD = mybir.AluOpType.add

    RHS_BLOCKS = 25  # 15 off + diag A + 8 yfT (shared A/B xy) + diag B
    lhsT_off_d = nc.dram_tensor(
        "lhsT_off", [128, GROUPS, OFF_UNITS, 128], bf16, kind="ExternalInput"
    )
    lhsT_run_d = nc.dram_tensor(
        "lhsT_run", [128, GROUPS, 2, 128], bf16, kind="ExternalInput"
    )
    rhs_d = nc.dram_tensor("rhs", [128, RHS_BLOCKS, 512], bf16, kind="ExternalInput")
    racc_d = nc.dram_tensor("racc", [128, GROUPS, 24, 128], bf16, kind="ExternalOutput")
    cs_d = nc.dram_tensor("cs", [128, GROUPS, 16, 128], bf16, kind="ExternalOutput")

    with tile.TileContext(nc) as tc:
        with (
            tc.tile_pool(name="consts", bufs=1) as consts,
            tc.tile_pool(name="accp", bufs=1) as accp,
            tc.tile_pool(name="scratch", bufs=3) as scratch_pool,
            tc.tile_pool(name="psum", bufs=2, space="PSUM") as psum_pool,
        ):
            # --- input tiles -------------------------------------------------
            rhs_splits = [(22, 24), (15, 19), (19, 22), (24, 25),
                          (0, 4), (8, 15), (4, 8)]
            rhs_tiles = {}
            for lo, hi in rhs_splits:
                rhs_tiles[lo] = consts.tile(
                    [128, hi - lo, 512], bf16, tag=f"rhs{lo}", name=f"rhs{lo}"
                )
            off_tiles = {}
            for s in range(GROUPS):
                off_tiles[s] = consts.tile(
                    [128, OFF_UNITS, 128], bf16,
                    tag=f"lhsToff{s}", name=f"lhsToff{s}",
                )
            run_tile = consts.tile(
                [128, GROUPS, 2, 128], bf16, tag="lhsTrun", name="lhsTrun"
            )

            # Input loads in strict consumption order.  The starter chunk's
            # rhs piece goes on the SCALAR engine's DGE queue (its only DMA)
            # so descriptor generation and the transfer run in parallel with
            # the sync queue's; everything else streams on sync.
            def rhs_dma(lo, eng=None):
                hi = dict(rhs_splits)[lo]
                (eng or nc.sync).dma_start(rhs_tiles[lo][:], rhs_d[:, lo:hi, :])

            rhs_dma(22, nc.scalar)
            nc.sync.dma_start(run_tile[:, 0], lhsT_run_d[:, 0])
            for lo, _hi in rhs_splits[1:4]:
                rhs_dma(lo)
            nc.sync.dma_start(off_tiles[0][:], lhsT_off_d[:, 0])
            for lo, _hi in rhs_splits[4:]:
                rhs_dma(lo)
            nc.sync.dma_start(run_tile[:, 1:], lhsT_run_d[:, 1:])
            for s in range(1, GROUPS):
                nc.sync.dma_start(off_tiles[s][:], lhsT_off_d[:, s])

            # ACT table warmup: dummy exp on a const tile, right after the
            # scalar queue's one d2d so the ~1.3us spline-table load hides
            # under the input DMA.
            ones_sb = consts.tile([128, 1], bf16, tag="ones")
            dummy_sb = consts.tile([128, 1], f32, tag="dummy")
            nc.vector.memset(ones_sb[:], 1.0)
            nc.scalar.activation(
                out=dummy_sb[:], in_=ones_sb[:],
                func=mybir.ActivationFunctionType.Exp,
            )

            def rhs_ap(slot):
                for lo, hi in rhs_splits:
                    if lo <= slot < hi:
                        return rhs_tiles[lo][:, slot - lo, :]
                raise AssertionError

            def lhsT_ap(s, u):
                kind, idx, _slot = _unit_aps(u)
                if kind == 0:
                    return run_tile[:, s, idx, :]
                return off_tiles[s][:, idx, :]

            # --- accumulators ------------------------------------------------
            # Per group: run chunks compress into racc [128,24,128] bf16
            # (c4 -> cols 0:8, c0 copy / c1..c3 add -> cols 8:24); off
            # chunks into cs [128,16,128] (c5 copy, c6..c8 add).  The host
            # computes rowsums and colsums from the shipped tiles.
            cs_tiles = [
                accp.tile([128, 16, 128], bf16, tag=f"cs{s}", name=f"cs{s}")
                for s in range(GROUPS)
            ]
            racc_tiles = [
                accp.tile([128, 24, 128], bf16, tag=f"racc{s}", name=f"racc{s}")
                for s in range(GROUPS)
            ]

            def do_chunk(s, j):
                units = _chunk_units(s, j)
                cols = 512 * len(units)
                n16 = cols // 128
                ps = psum_pool.tile([128, 2048], f32)
                for e, u in enumerate(units):
                    _kind, _idx, slot = _unit_aps(u)
                    nc.tensor.matmul(
                        ps[:, e * 512:(e + 1) * 512],
                        lhsT_ap(s, u),
                        rhs_ap(slot),
                        start=True,
                        stop=True,
                    )
                sc = scratch_pool.tile([128, 16, 128], bf16)
                nc.scalar.activation(
                    out=sc[:, :n16, :], in_=ps[:, :cols],
                    func=mybir.ActivationFunctionType.Exp,
                    scale=1.0 / TEMP,
                )
                cs = cs_tiles[s]
                src = sc[:, :n16, :]
                if j == 8:    # first off chunk executed: initializes cs[0:12]
                    nc.vector.tensor_copy(cs[:, :n16, :], src)
                elif j == 5:  # adds over c8's range, initializes the rest
                    nc.vector.tensor_tensor(
                        cs[:, :12, :], sc[:, :12, :], cs[:, :12, :], ADD
                    )
                    nc.vector.tensor_copy(cs[:, 12:16, :], sc[:, 12:16, :])
                elif j > 5:
                    nc.vector.tensor_tensor(cs[:, :n16, :], src, cs[:, :n16, :], ADD)
                elif j == 4:  # starter chunk: own slot in racc
                    nc.vector.tensor_copy(racc_tiles[s][:, 0:8, :], src)
                else:         # r4 chunk: shared racc slot, c0 initializes
                    dst = racc_tiles[s][:, 8:24, :]
                    if j == 0:
                        nc.vector.tensor_copy(dst, src)
                    else:
                        nc.vector.tensor_tensor(dst, src, dst, ADD)

            for s in range(GROUPS):
                # Chunk order is built so every chunk's PE refill hides
                # under the previous exp: c8 (3 matmuls) follows the short
                # starter/r4, full o4 chunks close each group, and group 3
                # front-loads its off chunks so the colsum writebacks hide
                # under compute.  c8 executes before c5 and initializes cs.
                if s == 0:
                    order = [4, 0, 1, 2, 3, 8, 5, 6, 7]
                elif s < GROUPS - 1:
                    order = [0, 1, 2, 3, 4, 8, 5, 6, 7]
                else:
                    order = [8, 5, 6, 7, 0, 1, 2, 3, 4]
                for j in order:
                    do_chunk(s, j)
                    if j == 7:
                        nc.sync.dma_start(cs_d[:, s], cs_tiles[s][:])
                    if (j == 3 and s == 0) or (j == 4 and 0 < s < GROUPS - 1):
                        nc.sync.dma_start(racc_d[:, s], racc_tiles[s][:])
                    if s == GROUPS - 1 and j == 3:
                        nc.sync.dma_start(
                            racc_d[:, s, 8:24], racc_tiles[s][:, 8:24, :]
                        )
                    if s == GROUPS - 1 and j == 4:
                        nc.sync.dma_start(
                            racc_d[:, s, 0:8], racc_tiles[s][:, 0:8, :]
                        )
    nc.compile()
    return nc


def _get_module():
    if "nc" not in _CACHED:
        _CACHED["nc"] = _build_module()
    return _CACHED["nc"]


def _core_off_blocks(k):
    """Off-diag (band, col) pairs for core k, in unit order 0..14."""
    A, B = k, (N_BANDS - 1) - k
    pairs = [(A, c) for c in range(A + 1, N_BANDS)]
    pairs += [(B, c) for c in range(B + 1, N_BANDS)]
    assert len(pairs) == OFF_UNITS
    return pairs


def _positive_terms(x64, yf64):
    """num_xy, G_diag, diag_self as float64 [512] vectors (exact math)."""
    xs = x64.reshape(N_BANDS, N_TRACKS, D)              # [16, 512, 128]
    yfs = yf64.reshape(NQ // N_TRACKS, N_TRACKS, D)     # [8, 512, 128]
    dxx = np.einsum("rid,cid->rci", xs, xs)             # [16, 16, 512]
    dxy = np.einsum("rid,qid->rqi", xs, yfs)            # [16, 8, 512]
    G = np.exp(dxx / TEMP).sum(axis=(0, 1))             # [512]
    diag_self = np.exp(np.einsum("rid,rid->ri", xs, xs) / TEMP).sum(axis=0)
    num_xy = np.exp(dxy / TEMP).sum(axis=(0, 1))        # [512]
    return num_xy, G, diag_self


def _finish(rs_seg, num_xy, G, diag_self):
    num = num_xy + (G - diag_self) / 2.0
    den = rs_seg - num_xy - G
    loss = np.mean(-np.log(num / (den + num))) / Q
    return np.asarray(loss, dtype=np.float32)


def _numpy_fallback(x, track_idxs, y):
    """Exact general-track reference in numpy (safety net only)."""
    x64 = x.astype(np.float64)
    yf64 = y.reshape(NQ, D).astype(np.float64)
    t = track_idxs.astype(np.int64)
    y_idxs = np.tile(np.arange(N_TRACKS, dtype=np.int64), Q)
    E_xy = np.exp(x64 @ yf64.T / TEMP)
    Sx = np.zeros((N_TRACKS, NQ))
    np.add.at(Sx, t, E_xy)
    Py = (y_idxs[:, None] == np.arange(N_TRACKS)[None, :]).astype(np.float64)
    num_xy = np.einsum("ik,ki->i", Sx, Py)
    den_xy = Sx.sum(axis=1) - num_xy
    E_xx = np.exp(x64 @ x64.T / TEMP)
    Sxx = np.zeros((N_TRACKS, M))
    np.add.at(Sxx, t, E_xx)
    Px = (t[:, None] == np.arange(N_TRACKS)[None, :]).astype(np.float64)
    G_diag = np.einsum("im,mi->i", Sxx, Px)
    diag_self = np.zeros(N_TRACKS)
    np.add.at(diag_self, t, np.diagonal(E_xx))
    num_xx = (G_diag - diag_self) / 2.0
    den_xx = Sxx.sum(axis=1) - G_diag
    num = num_xy + num_xx
    den = den_xy + den_xx
    loss = np.mean(-np.log(num / (den + num))) / Q
    return np.asarray(loss, dtype=np.float32)


def kernel(x, track_idxs, y):
    x = np.asarray(x, dtype=np.float32)
    y = np.asarray(y, dtype=np.float32)
    track_idxs = np.asarray(track_idxs)

    expected_tracks = np.arange(M, dtype=np.int64) % N_TRACKS
    if (
        x.shape != (M, D)
        or y.shape != (N_TRACKS, Q, D)
        or not np.array_equal(track_idxs.astype(np.int64), expected_tracks)
    ):
        return _numpy_fallback(x, track_idxs, y)

    from concourse.bass_utils import run_bass_kernel_spmd

    yf = np.ascontiguousarray(y.reshape(NQ, D))
    xT = np.ascontiguousarray(x.T).astype(ml_dtypes.bfloat16)    # [128, 8192]
    yfT = np.ascontiguousarray(yf.T).astype(ml_dtypes.bfloat16)  # [128, 4096]
    xT_blocks = xT.reshape(128, N_BANDS, 512)
    yfT_blocks = yfT.reshape(128, Q, 512)

    in_maps = []
    for k in range(N_CORES):
        A, B = k, (N_BANDS - 1) - k
        pairs = _core_off_blocks(k)
        rhs = np.empty((128, 25, 512), dtype=ml_dtypes.bfloat16)
        for u, (_band, c) in enumerate(pairs):
            rhs[:, u] = xT_blocks[:, c]
        rhs[:, 15] = xT_blocks[:, A]
        rhs[:, 16:24] = yfT_blocks
        rhs[:, 24] = xT_blocks[:, B]

        def subtile(band, s):
            t = 4 * band + s
            return xT[:, 128 * t: 128 * (t + 1)]

        lhsT_off = np.empty((128, GROUPS, OFF_UNITS, 128), dtype=ml_dtypes.bfloat16)
        lhsT_run = np.empty((128, GROUPS, 2, 128), dtype=ml_dtypes.bfloat16)
        for s in range(GROUPS):
            for u, (band, _c) in enumerate(pairs):
                lhsT_off[:, s, u, :] = subtile(band, s)
            lhsT_run[:, s, 0, :] = subtile(A, s)
            lhsT_run[:, s, 1, :] = subtile(B, s)
        in_maps.append(
            {
                "lhsT_off": np.ascontiguousarray(lhsT_off),
                "lhsT_run": np.ascontiguousarray(lhsT_run),
                "rhs": np.ascontiguousarray(rhs),
            }
        )

    nc = _get_module()
    res = run_bass_kernel_spmd(nc, in_maps, core_ids=list(range(N_CORES)))
    _CACHED["last_res"] = res

    # Fold rowsums by row residue (128s + p) and mirror colsums by in-block
    # column position (mod 512) -- the "all-reduce" -- on the host.
    rs_seg = np.zeros(N_TRACKS, dtype=np.float64)
    for k in range(N_CORES):
        racc = np.asarray(res.results[k]["racc"], dtype=np.float64)
        cs = np.asarray(res.results[k]["cs"], dtype=np.float64)
        # racc [128, 4, 24*128], cs [128, 4, 16*128]: rowsums per (p, s)
        per_group = racc.reshape(128, GROUPS, -1).sum(axis=2)
        per_group += cs.reshape(128, GROUPS, -1).sum(axis=2)
        rs_seg += per_group.T.reshape(N_TRACKS)  # i = 128*s + p
        # colsums: every 512-col block of cs is one rhs block; positions
        # fold mod 512
        rs_seg += cs.reshape(-1, 4, 512).sum(axis=(0, 1))

    num_xy, G, diag_self = _positive_terms(
        x.astype(np.float64), yf.astype(np.float64)
    )
    return _finish(rs_seg, num_xy, G, diag_self)


# revision 34
# speedup vs baseline: 1.1071x; 1.0048x over previous
"""Trainium2 Bass kernel for nn_ContrastiveLoss_76501957477132.

Math (see reference): with T=0.3, n=512 tracks, Q=8, M=8192, D=128,
  yf = y.reshape(nQ, D), y_idxs[k] = k % n, track_idxs[a] = a % n.
Per track i:
  num_xy[i] = sum_{a=i mod n} sum_{k=i mod n} exp(x_a.yf_k/T)
  den_xy[i] = sum_{a=i mod n} sum_k exp(x_a.yf_k/T) - num_xy[i]
  G[i]      = sum_{a=i mod n} sum_{m=i mod n} exp(x_a.x_m/T)
  num_xx[i] = (G[i] - diag_self[i]) / 2
  den_xx[i] = sum_{a=i mod n} sum_m exp(x_a.x_m/T) - G[i]
  loss = mean(-log(num/(num+den))) / Q

Track labels are (row index mod 512), so the device only needs
residue-class sums of exp over the E_xx (symmetric; upper triangle
only, colsums complete the mirrored rows) and E_xy matrices.
Positive-pair terms (tiny) are exact on the host in float64.

Work per core: 4 residue groups (s) x 33 [128x512] matmul units via
band pairing (A=k, B=15-k): 18 run units (2 diag + 16 xy) + 15 off.

v4 pipeline -- the kernel is ACT(exp)-bound, 56.3us of exp columns
is the floor, so ScalarE does *nothing but* 36 big exp instructions:
- PSUM 2 x [128,2048] fp32 (all 8 banks); chunks of <=4 units.
- Chunks per group: 4xr4, r2 (run), 3xo4, o3 (off).  The r2 chunk
  runs first (fastest start); each group ends with the o3 chunk
  (1536 cols covers the next group's PE refill, no ACT bubble).
- VectorE tensor_reduce has no 2x uop variant (measured ~2.2us per
  [128,2048] regardless of dtype), so the device reduces NOTHING:
  every chunk's exp scratch is compressed into per-group bf16
  accumulator tiles with tensor_copy / in-place tensor_tensor adds
  only (4x / 2x perf modes, ~0.6-1.2us per chunk), run and off
  chunks into separate tiles.  All 8 tiles ship to DRAM and the
  host folds rowsums (run+off tiles) and mirror colsums (off tiles,
  positions mod 512) in float64.
- Group 3 runs its off chunks first and its r2 chunk last so both
  final writebacks hide under compute.
- ACT spline tables are warmed by a dummy exp at t=0.  No gpsimd
  anywhere (its DGE drain lengthens the end-of-kernel barrier).
"""

import numpy as np
import ml_dtypes

M, D, N_TRACKS, Q = 8192, 128, 512, 8
NQ = N_TRACKS * Q  # 4096
TEMP = 0.3
N_CORES = 8
N_BANDS = M // N_TRACKS           # 16 row/col bands of 512
GROUPS = 4                        # residue groups (s): rows 128s..128s+127 of a band
UNITS = 33                        # units per group: 18 run (2 diag + 16 xy) + 15 off
RUN_UNITS = 18
OFF_UNITS = 15
CHUNKS = 9                        # c0..c3 r4, c4 r2, c5..c7 o4, c8 o3

_CACHED = {}


def _unit_aps(u):
    """(lhsT_kind, lhsT_idx, rhs_slot) for unit u of a group.

    lhsT_kind 0 -> 128-col slice s of diag block A/B (idx 0/1) in rhs;
    kind 1 -> lhsT_off[:, s, idx].
    Unit order: diag A, A-xy q0..7, diag B, B-xy q0..7, off 0..14.
    rhs slots: 0..14 off blocks, 15 diag A, 16..23 yfT, 24 diag B.
    """
    if u == 0:
        return (0, 0, 15)
    if 1 <= u <= 8:
        return (0, 0, 15 + u)
    if u == 9:
        return (0, 1, 24)
    if 10 <= u <= 17:
        return (0, 1, 6 + u)
    return (1, u - RUN_UNITS, u - RUN_UNITS)


def _chunk_units(s, j):
    """Units of chunk j in group s (c4 = 2-unit starter, c8 = 3-unit)."""
    del s
    if j == 4:
        return [16, 17]
    if j == 8:
        return [30, 31, 32]
    return list(range(4 * j, 4 * j + 4)) if j < 4 else list(range(4 * j - 2, 4 * j + 2))


def _build_module():
    import concourse.bacc as bacc
    import concourse.tile as tile
    import concourse.mybir as mybir

    nc = bacc.Bacc(None, target_bir_lowering=False)
    bf16 = mybir.dt.bfloat16
    f32 = mybir.dt.float32
    ADD = mybir.AluOpType.add

    RHS_BLOCKS = 25  # 15 off + diag A + 8 yfT (shared A/B xy) + diag B
    lhsT_off_d = nc.dram_tensor(
        "lhsT_off", [128, GROUPS, OFF_UNITS, 128], bf16, kind="ExternalInput"
    )
    rhs_d = nc.dram_tensor("rhs", [128, RHS_BLOCKS, 512], bf16, kind="ExternalInput")
    racc_d = nc.dram_tensor("racc", [128, GROUPS, 24, 128], bf16, kind="ExternalOutput")
    cs_d = nc.dram_tensor("cs", [128, GROUPS, 16, 128], bf16, kind="ExternalOutput")

    with tile.TileContext(nc) as tc:
        with (
            tc.tile_pool(name="consts", bufs=1) as consts,
            tc.tile_pool(name="accp", bufs=1) as accp,
            tc.tile_pool(name="scratch", bufs=4) as scratch_pool,
            tc.tile_pool(name="psum", bufs=2, space="PSUM") as psum_pool,
        ):
            # --- input tiles -------------------------------------------------
            # Moderate piece sizes: the DMA path issues one descriptor per
            # partition per piece, so pieces batch several blocks -- but not
            # so many that a fat piece monopolizes the engines while the
            # early chunks wait.
            rhs_splits = [(22, 25), (15, 19), (19, 22), (0, 8), (8, 15)]
            rhs_tiles = {}
            for lo, hi in rhs_splits:
                rhs_tiles[lo] = consts.tile(
                    [128, hi - lo, 512], bf16, tag=f"rhs{lo}", name=f"rhs{lo}"
                )
            off_tiles = {
                0: consts.tile([128, OFF_UNITS, 128], bf16,
                               tag="lhsToff0", name="lhsToff0"),
                1: consts.tile([128, GROUPS - 1, OFF_UNITS, 128], bf16,
                               tag="lhsToff123", name="lhsToff123"),
            }

            # Consumption order: the starter chunk's rhs piece goes on the
            # SCALAR engine's DGE queue (its only DMA) so its descriptors
            # and transfer run in parallel with the sync queue's.
            def rhs_dma(lo, eng=None):
                hi = dict(rhs_splits)[lo]
                (eng or nc.sync).dma_start(rhs_tiles[lo][:], rhs_d[:, lo:hi, :])

            rhs_dma(22, nc.scalar)
            rhs_dma(15)
            rhs_dma(19)
            nc.sync.dma_start(off_tiles[0][:], lhsT_off_d[:, 0])
            rhs_dma(0)
            rhs_dma(8)
            nc.sync.dma_start(off_tiles[1][:], lhsT_off_d[:, 1:])

            # ACT table warmup: dummy exp on a const tile, right after the
            # scalar queue's one d2d so the ~1.3us spline-table load hides
            # under the input DMA.
            ones_sb = consts.tile([128, 1], bf16, tag="ones")
            dummy_sb = consts.tile([128, 1], f32, tag="dummy")
            nc.vector.memset(ones_sb[:], 1.0)
            nc.scalar.activation(
                out=dummy_sb[:], in_=ones_sb[:],
                func=mybir.ActivationFunctionType.Exp,
            )

            def rhs_ap(slot):
                for lo, hi in rhs_splits:
                    if lo <= slot < hi:
                        return rhs_tiles[lo][:, slot - lo, :]
                raise AssertionError

            def lhsT_ap(s, u):
                kind, idx, _slot = _unit_aps(u)
                if kind == 0:
                    # run-unit lhsT slices live inside the diag rhs blocks
                    diag = rhs_ap(15 if idx == 0 else 24)
                    return diag[:, 128 * s: 128 * (s + 1)]
                if s == 0:
                    return off_tiles[0][:, idx, :]
                return off_tiles[1][:, s - 1, idx, :]

            # --- accumulators ------------------------------------------------
            # Per group: run chunks compress into racc [128,24,128] bf16
            # (c4 -> cols 0:8, c0 copy / c1..c3 add -> cols 8:24); off
            # chunks into cs [128,16,128] (c5 copy, c6..c8 add).  The host
            # computes rowsums and colsums from the shipped tiles.
            cs_tiles = [
                accp.tile([128, 16, 128], bf16, tag=f"cs{s}", name=f"cs{s}")
                for s in range(GROUPS)
            ]
            racc_tiles = [
                accp.tile([128, 24, 128], bf16, tag=f"racc{s}", name=f"racc{s}")
                for s in range(GROUPS)
            ]

            def do_chunk(s, j):
                units = _chunk_units(s, j)
                cols = 512 * len(units)
                n16 = cols // 128
                ps = psum_pool.tile([128, 2048], f32)
                for e, u in enumerate(units):
                    _kind, _idx, slot = _unit_aps(u)
                    nc.tensor.matmul(
                        ps[:, e * 512:(e + 1) * 512],
                        lhsT_ap(s, u),
                        rhs_ap(slot),
                        start=True,
                        stop=True,
                    )
                sc = scratch_pool.tile([128, 16, 128], bf16)
                nc.scalar.activation(
                    out=sc[:, :n16, :], in_=ps[:, :cols],
                    func=mybir.ActivationFunctionType.Exp,
                    scale=1.0 / TEMP,
                )
                cs = cs_tiles[s]
                src = sc[:, :n16, :]
                if j == 8:    # first off chunk executed: initializes cs[0:12]
                    nc.vector.tensor_copy(cs[:, :n16, :], src)
                elif j == 5:  # adds over c8's range, initializes the rest
                    nc.vector.tensor_tensor(
                        cs[:, :12, :], sc[:, :12, :], cs[:, :12, :], ADD
                    )
                    nc.vector.tensor_copy(cs[:, 12:16, :], sc[:, 12:16, :])
                elif j > 5:
                    nc.vector.tensor_tensor(cs[:, :n16, :], src, cs[:, :n16, :], ADD)
                elif j == 4:  # starter chunk: own slot in racc
                    nc.vector.tensor_copy(racc_tiles[s][:, 0:8, :], src)
                else:         # r4 chunk: shared racc slot, c0 initializes
                    dst = racc_tiles[s][:, 8:24, :]
                    if j == 0:
                        nc.vector.tensor_copy(dst, src)
                    else:
                        nc.vector.tensor_tensor(dst, src, dst, ADD)

            for s in range(GROUPS):
                # Chunk order is built so every chunk's PE refill hides
                # under the previous exp: c8 (3 matmuls) follows the short
                # starter/r4, full o4 chunks close each group, and group 3
                # front-loads its off chunks so the colsum writebacks hide
                # under compute.  c8 executes before c5 and initializes cs.
                if s == 0:
                    order = [4, 0, 1, 2, 3, 8, 5, 6, 7]
                else:
                    order = [0, 1, 2, 3, 4, 8, 5, 6, 7]
                for j in order:
                    do_chunk(s, j)
                    if j == 7:
                        nc.sync.dma_start(cs_d[:, s], cs_tiles[s][:])
                    if (j == 3 and s == 0) or (j == 4 and s > 0):
                        nc.sync.dma_start(racc_d[:, s], racc_tiles[s][:])
    nc.compile()
    return nc


def _get_module():
    if "nc" not in _CACHED:
        _CACHED["nc"] = _build_module()
    return _CACHED["nc"]


def _core_off_blocks(k):
    """Off-diag (band, col) pairs for core k, in unit order 0..14."""
    A, B = k, (N_BANDS - 1) - k
    pairs = [(A, c) for c in range(A + 1, N_BANDS)]
    pairs += [(B, c) for c in range(B + 1, N_BANDS)]
    assert len(pairs) == OFF_UNITS
    return pairs


def _positive_terms(x64, yf64):
    """num_xy, G_diag, diag_self as float64 [512] vectors (exact math)."""
    xs = x64.reshape(N_BANDS, N_TRACKS, D)              # [16, 512, 128]
    yfs = yf64.reshape(NQ // N_TRACKS, N_TRACKS, D)     # [8, 512, 128]
    dxx = np.einsum("rid,cid->rci", xs, xs)             # [16, 16, 512]
    dxy = np.einsum("rid,qid->rqi", xs, yfs)            # [16, 8, 512]
    G = np.exp(dxx / TEMP).sum(axis=(0, 1))             # [512]
    diag_self = np.exp(np.einsum("rid,rid->ri", xs, xs) / TEMP).sum(axis=0)
    num_xy = np.exp(dxy / TEMP).sum(axis=(0, 1))        # [512]
    return num_xy, G, diag_self


def _finish(rs_seg, num_xy, G, diag_self):
    num = num_xy + (G - diag_self) / 2.0
    den = rs_seg - num_xy - G
    loss = np.mean(-np.log(num / (den + num))) / Q
    return np.asarray(loss, dtype=np.float32)


def _numpy_fallback(x, track_idxs, y):
    """Exact general-track reference in numpy (safety net only)."""
    x64 = x.astype(np.float64)
    yf64 = y.reshape(NQ, D).astype(np.float64)
    t = track_idxs.astype(np.int64)
    y_idxs = np.tile(np.arange(N_TRACKS, dtype=np.int64), Q)
    E_xy = np.exp(x64 @ yf64.T / TEMP)
    Sx = np.zeros((N_TRACKS, NQ))
    np.add.at(Sx, t, E_xy)
    Py = (y_idxs[:, None] == np.arange(N_TRACKS)[None, :]).astype(np.float64)
    num_xy = np.einsum("ik,ki->i", Sx, Py)
    den_xy = Sx.sum(axis=1) - num_xy
    E_xx = np.exp(x64 @ x64.T / TEMP)
    Sxx = np.zeros((N_TRACKS, M))
    np.add.at(Sxx, t, E_xx)
    Px = (t[:, None] == np.arange(N_TRACKS)[None, :]).astype(np.float64)
    G_diag = np.einsum("im,mi->i", Sxx, Px)
    diag_self = np.zeros(N_TRACKS)
    np.add.at(diag_self, t, np.diagonal(E_xx))
    num_xx = (G_diag - diag_self) / 2.0
    den_xx = Sxx.sum(axis=1) - G_diag
    num = num_xy + num_xx
    den = den_xy + den_xx
    loss = np.mean(-np.log(num / (den + num))) / Q
    return np.asarray(loss, dtype=np.float32)


def kernel(x, track_idxs, y):
    x = np.asarray(x, dtype=np.float32)
    y = np.asarray(y, dtype=np.float32)
    track_idxs = np.asarray(track_idxs)

    expected_tracks = np.arange(M, dtype=np.int64) % N_TRACKS
    if (
        x.shape != (M, D)
        or y.shape != (N_TRACKS, Q, D)
        or not np.array_equal(track_idxs.astype(np.int64), expected_tracks)
    ):
        return _numpy_fallback(x, track_idxs, y)

    from concourse.bass_utils import run_bass_kernel_spmd

    yf = np.ascontiguousarray(y.reshape(NQ, D))
    xT = np.ascontiguousarray(x.T).astype(ml_dtypes.bfloat16)    # [128, 8192]
    yfT = np.ascontiguousarray(yf.T).astype(ml_dtypes.bfloat16)  # [128, 4096]
    xT_blocks = xT.reshape(128, N_BANDS, 512)
    yfT_blocks = yfT.reshape(128, Q, 512)

    in_maps = []
    for k in range(N_CORES):
        A, B = k, (N_BANDS - 1) - k
        pairs = _core_off_blocks(k)
        rhs = np.empty((128, 25, 512), dtype=ml_dtypes.bfloat16)
        for u, (_band, c) in enumerate(pairs):
            rhs[:, u] = xT_blocks[:, c]
        rhs[:, 15] = xT_blocks[:, A]
        rhs[:, 16:24] = yfT_blocks
        rhs[:, 24] = xT_blocks[:, B]

        def subtile(band, s):
            t = 4 * band + s
            return xT[:, 128 * t: 128 * (t + 1)]

        lhsT_off = np.empty((128, GROUPS, OFF_UNITS, 128), dtype=ml_dtypes.bfloat16)
        for s in range(GROUPS):
            for u, (band, _c) in enumerate(pairs):
                lhsT_off[:, s, u, :] = subtile(band, s)
        in_maps.append(
            {
                "lhsT_off": np.ascontiguousarray(lhsT_off),
                "rhs": np.ascontiguousarray(rhs),
            }
        )

    nc = _get_module()
    res = run_bass_kernel_spmd(nc, in_maps, core_ids=list(range(N_CORES)))
    _CACHED["last_res"] = res

    # Fold rowsums by row residue (128s + p) and mirror colsums by in-block
    # column position (mod 512) -- the "all-reduce" -- on the host.
    rs_seg = np.zeros(N_TRACKS, dtype=np.float64)
    for k in range(N_CORES):
        racc = np.asarray(res.results[k]["racc"], dtype=np.float64)
        cs = np.asarray(res.results[k]["cs"], dtype=np.float64)
        # racc [128, 4, 24*128], cs [128, 4, 16*128]: rowsums per (p, s)
        per_group = racc.reshape(128, GROUPS, -1).sum(axis=2)
        per_group += cs.reshape(128, GROUPS, -1).sum(axis=2)
        rs_seg += per_group.T.reshape(N_TRACKS)  # i = 128*s + p
        # colsums: every 512-col block of cs is one rhs block; positions
        # fold mod 512
        rs_seg += cs.reshape(-1, 4, 512).sum(axis=(0, 1))

    num_xy, G, diag_self = _positive_terms(
        x.astype(np.float64), yf.astype(np.float64)
    )
    return _finish(rs_seg, num_xy, G, diag_self)


# revision 37
# speedup vs baseline: 1.1132x; 1.0055x over previous
"""Trainium2 Bass kernel for nn_ContrastiveLoss_76501957477132.

Math (see reference): with T=0.3, n=512 tracks, Q=8, M=8192, D=128,
  yf = y.reshape(nQ, D), y_idxs[k] = k % n, track_idxs[a] = a % n.
Per track i:
  num_xy[i] = sum_{a=i mod n} sum_{k=i mod n} exp(x_a.yf_k/T)
  den_xy[i] = sum_{a=i mod n} sum_k exp(x_a.yf_k/T) - num_xy[i]
  G[i]      = sum_{a=i mod n} sum_{m=i mod n} exp(x_a.x_m/T)
  num_xx[i] = (G[i] - diag_self[i]) / 2
  den_xx[i] = sum_{a=i mod n} sum_m exp(x_a.x_m/T) - G[i]
  loss = mean(-log(num/(num+den))) / Q

Track labels are (row index mod 512), so the device only needs
residue-class sums of exp over the E_xx (symmetric; upper triangle
only, colsums complete the mirrored rows) and E_xy matrices.
Positive-pair terms (tiny) are exact on the host in float64.

Work per core: 4 residue groups (s) x 33 [128x512] matmul units via
band pairing (A=k, B=15-k): 18 run units (2 diag + 16 xy) + 15 off.

v4 pipeline -- the kernel is ACT(exp)-bound, 56.3us of exp columns
is the floor, so ScalarE does *nothing but* 36 big exp instructions:
- PSUM 2 x [128,2048] fp32 (all 8 banks); chunks of <=4 units.
- Chunks per group: 4xr4, r2 (run), 3xo4, o3 (off).  The r2 chunk
  runs first (fastest start); each group ends with the o3 chunk
  (1536 cols covers the next group's PE refill, no ACT bubble).
- VectorE tensor_reduce has no 2x uop variant (measured ~2.2us per
  [128,2048] regardless of dtype), so the device reduces NOTHING:
  every chunk's exp scratch is compressed into per-group bf16
  accumulator tiles with tensor_copy / in-place tensor_tensor adds
  only (4x / 2x perf modes, ~0.6-1.2us per chunk), run and off
  chunks into separate tiles.  All 8 tiles ship to DRAM and the
  host folds rowsums (run+off tiles) and mirror colsums (off tiles,
  positions mod 512) in float64.
- Group 3 runs its off chunks first and its r2 chunk last so both
  final writebacks hide under compute.
- ACT spline tables are warmed by a dummy exp at t=0.  No gpsimd
  anywhere (its DGE drain lengthens the end-of-kernel barrier).
"""

import numpy as np
import ml_dtypes

M, D, N_TRACKS, Q = 8192, 128, 512, 8
NQ = N_TRACKS * Q  # 4096
TEMP = 0.3
N_CORES = 8
N_BANDS = M // N_TRACKS           # 16 row/col bands of 512
GROUPS = 4                        # residue groups (s): rows 128s..128s+127 of a band
UNITS = 33                        # units per group: 18 run (2 diag + 16 xy) + 15 off
RUN_UNITS = 18
OFF_UNITS = 15
CHUNKS = 9                        # c0..c3 r4, c4 r2, c5..c7 o4, c8 o3

_CACHED = {}


def _unit_aps(u):
    """(lhsT_kind, lhsT_idx, rhs_slot) for unit u of a group.

    lhsT_kind 0 -> 128-col slice s of diag block A/B (idx 0/1) in rhs;
    kind 1 -> lhsT_off[:, s, idx].
    Unit order: diag A, A-xy q0..7, diag B, B-xy q0..7, off 0..14.
    rhs slots: 0..14 off blocks, 15 diag A, 16..23 yfT, 24 diag B.
    """
    if u == 0:
        return (0, 0, 15)
    if 1 <= u <= 8:
        return (0, 0, 15 + u)
    if u == 9:
        return (0, 1, 24)
    if 10 <= u <= 17:
        return (0, 1, 6 + u)
    return (1, u - RUN_UNITS, u - RUN_UNITS)


def _chunk_units(s, j):
    """Units of chunk j in group s (c4 = 2-unit starter, c8 = 3-unit)."""
    del s
    if j == 4:
        return [16, 17]
    if j == 8:
        return [30, 31, 32]
    return list(range(4 * j, 4 * j + 4)) if j < 4 else list(range(4 * j - 2, 4 * j + 2))


def _build_module():
    import concourse.bacc as bacc
    import concourse.tile as tile
    import concourse.mybir as mybir

    nc = bacc.Bacc(None, target_bir_lowering=False)
    bf16 = mybir.dt.bfloat16
    f32 = mybir.dt.float32
    ADD = mybir.AluOpType.add

    RHS_BLOCKS = 25  # 15 off + diag A + 8 yfT (shared A/B xy) + diag B
    lhsT_off_d = nc.dram_tensor(
        "lhsT_off", [128, GROUPS, OFF_UNITS, 128], bf16, kind="ExternalInput"
    )
    rhs_d = nc.dram_tensor("rhs", [128, RHS_BLOCKS, 512], bf16, kind="ExternalInput")
    racc_d = nc.dram_tensor("racc", [128, GROUPS, 24, 128], bf16, kind="ExternalOutput")
    cs_d = nc.dram_tensor("cs", [128, GROUPS, 16, 128], bf16, kind="ExternalOutput")

    with tile.TileContext(nc) as tc:
        with (
            tc.tile_pool(name="consts", bufs=1) as consts,
            tc.tile_pool(name="accp", bufs=1) as accp,
            tc.tile_pool(name="scratch", bufs=4) as scratch_pool,
            tc.tile_pool(name="psum", bufs=2, space="PSUM") as psum_pool,
        ):
            # --- input tiles -------------------------------------------------
            # Moderate piece sizes: the DMA path issues one descriptor per
            # partition per piece, so pieces batch several blocks -- but not
            # so many that a fat piece monopolizes the engines while the
            # early chunks wait.
            rhs_splits = [(22, 25), (15, 19), (19, 22), (0, 8), (8, 15)]
            rhs_tiles = {}
            for lo, hi in rhs_splits:
                rhs_tiles[lo] = consts.tile(
                    [128, hi - lo, 512], bf16, tag=f"rhs{lo}", name=f"rhs{lo}"
                )
            off_tiles = {
                0: consts.tile([128, OFF_UNITS, 128], bf16,
                               tag="lhsToff0", name="lhsToff0"),
                1: consts.tile([128, GROUPS - 1, OFF_UNITS, 128], bf16,
                               tag="lhsToff123", name="lhsToff123"),
            }

            # Consumption order: the starter chunk's rhs piece goes on the
            # SCALAR engine's DGE queue (its only DMA) so its descriptors
            # and transfer run in parallel with the sync queue's.
            def rhs_dma(lo, eng=None):
                hi = dict(rhs_splits)[lo]
                (eng or nc.sync).dma_start(rhs_tiles[lo][:], rhs_d[:, lo:hi, :])

            rhs_dma(22, nc.scalar)
            rhs_dma(15)
            rhs_dma(19)
            nc.sync.dma_start(off_tiles[0][:], lhsT_off_d[:, 0])

            # ACT table warmup: dummy exp on a const tile, right after the
            # scalar queue's one d2d so the ~1.3us spline-table load hides
            # under the input DMA.
            ones_sb = consts.tile([128, 1], bf16, tag="ones")
            dummy_sb = consts.tile([128, 1], f32, tag="dummy")
            nc.vector.memset(ones_sb[:], 1.0)
            nc.scalar.activation(
                out=dummy_sb[:], in_=ones_sb[:],
                func=mybir.ActivationFunctionType.Exp,
            )

            def rhs_ap(slot):
                for lo, hi in rhs_splits:
                    if lo <= slot < hi:
                        return rhs_tiles[lo][:, slot - lo, :]
                raise AssertionError

            def lhsT_ap(s, u):
                kind, idx, _slot = _unit_aps(u)
                if kind == 0:
                    # run-unit lhsT slices live inside the diag rhs blocks
                    diag = rhs_ap(15 if idx == 0 else 24)
                    return diag[:, 128 * s: 128 * (s + 1)]
                if s == 0:
                    return off_tiles[0][:, idx, :]
                return off_tiles[1][:, s - 1, idx, :]

            # --- accumulators ------------------------------------------------
            # Per group: run chunks compress into racc [128,24,128] bf16
            # (c4 -> cols 0:8, c0 copy / c1..c3 add -> cols 8:24); off
            # chunks into cs [128,16,128] (c5 copy, c6..c8 add).  The host
            # computes rowsums and colsums from the shipped tiles.
            cs_tiles = [
                accp.tile([128, 16, 128], bf16, tag=f"cs{s}", name=f"cs{s}")
                for s in range(GROUPS)
            ]
            racc_tiles = [
                accp.tile([128, 24, 128], bf16, tag=f"racc{s}", name=f"racc{s}")
                for s in range(GROUPS)
            ]

            def do_chunk(s, j):
                units = _chunk_units(s, j)
                cols = 512 * len(units)
                n16 = cols // 128
                ps = psum_pool.tile([128, 2048], f32)
                for e, u in enumerate(units):
                    _kind, _idx, slot = _unit_aps(u)
                    nc.tensor.matmul(
                        ps[:, e * 512:(e + 1) * 512],
                        lhsT_ap(s, u),
                        rhs_ap(slot),
                        start=True,
                        stop=True,
                    )
                sc = scratch_pool.tile([128, 16, 128], bf16)
                nc.scalar.activation(
                    out=sc[:, :n16, :], in_=ps[:, :cols],
                    func=mybir.ActivationFunctionType.Exp,
                    scale=1.0 / TEMP,
                )
                cs = cs_tiles[s]
                src = sc[:, :n16, :]
                last = s == GROUPS - 1
                if j >= 5:
                    # cs init: groups 0-2 execute c8 first (copy), then c5
                    # adds c8's range and initializes the rest; group 3 runs
                    # c5 first (full copy) and ends on c8 (add)
                    if (j == 8 and not last) or (j == 5 and last):
                        nc.vector.tensor_copy(cs[:, :n16, :], src)
                    elif j == 5:
                        nc.vector.tensor_tensor(
                            cs[:, :12, :], sc[:, :12, :], cs[:, :12, :], ADD
                        )
                        nc.vector.tensor_copy(cs[:, 12:16, :], sc[:, 12:16, :])
                    else:
                        nc.vector.tensor_tensor(
                            cs[:, :n16, :], src, cs[:, :n16, :], ADD
                        )
                elif j == 4:  # starter chunk: own slot in racc
                    nc.vector.tensor_copy(racc_tiles[s][:, 0:8, :], src)
                else:         # r4 chunk: shared racc slot, c0 initializes
                    dst = racc_tiles[s][:, 8:24, :]
                    if j == 0:
                        nc.vector.tensor_copy(dst, src)
                    else:
                        nc.vector.tensor_tensor(dst, src, dst, ADD)

            for s in range(GROUPS):
                # Chunk order is built so every chunk's PE refill hides
                # under the previous exp: c8 (3 matmuls) follows the short
                # starter/r4, full o4 chunks close each group, and group 3
                # front-loads its off chunks so the colsum writebacks hide
                # under compute.  c8 executes before c5 and initializes cs.
                if s == 0:
                    order = [4, 0, 1, 2, 3, 8, 5, 6, 7]
                elif s < GROUPS - 1:
                    order = [0, 1, 2, 3, 4, 8, 5, 6, 7]
                else:
                    order = [0, 1, 2, 3, 4, 5, 6, 7, 8]
                for j in order:
                    do_chunk(s, j)
                    if s == 0 and j == 4:
                        # bulk input pieces wait for the ramp: a 1-element
                        # memset on each destination tile (vector stream,
                        # after the starter chunk) makes their DMAs queue
                        # behind it, so the 8 cores' ~4MB bulk loads don't
                        # starve each other's startup pieces chip-wide.
                        nc.vector.memset(rhs_tiles[0][:, 0, 0:1], 0.0)
                        nc.vector.memset(rhs_tiles[8][:, 0, 0:1], 0.0)
                        nc.vector.memset(off_tiles[1][:, 0, 0, 0:1], 0.0)
                        rhs_dma(0)
                        rhs_dma(8)
                        nc.sync.dma_start(off_tiles[1][:], lhsT_off_d[:, 1:])
                    if s < GROUPS - 1:
                        if j == 7:
                            nc.sync.dma_start(cs_d[:, s], cs_tiles[s][:])
                        if (j == 3 and s == 0) or (j == 4 and s > 0):
                            nc.sync.dma_start(racc_d[:, s], racc_tiles[s][:])
                    else:
                        if j == 4:
                            nc.sync.dma_start(racc_d[:, s], racc_tiles[s][:])
                        if j == 7:
                            nc.sync.dma_start(
                                cs_d[:, s, 12:16], cs_tiles[s][:, 12:16, :]
                            )
                        if j == 8:
                            nc.sync.dma_start(
                                cs_d[:, s, 0:12], cs_tiles[s][:, 0:12, :]
                            )
    nc.compile()
    return nc


def _get_module():
    if "nc" not in _CACHED:
        _CACHED["nc"] = _build_module()
    return _CACHED["nc"]


def _core_off_blocks(k):
    """Off-diag (band, col) pairs for core k, in unit order 0..14."""
    A, B = k, (N_BANDS - 1) - k
    pairs = [(A, c) for c in range(A + 1, N_BANDS)]
    pairs += [(B, c) for c in range(B + 1, N_BANDS)]
    assert len(pairs) == OFF_UNITS
    return pairs


def _positive_terms(x64, yf64):
    """num_xy, G_diag, diag_self as float64 [512] vectors (exact math)."""
    xs = x64.reshape(N_BANDS, N_TRACKS, D)              # [16, 512, 128]
    yfs = yf64.reshape(NQ // N_TRACKS, N_TRACKS, D)     # [8, 512, 128]
    dxx = np.einsum("rid,cid->rci", xs, xs)             # [16, 16, 512]
    dxy = np.einsum("rid,qid->rqi", xs, yfs)            # [16, 8, 512]
    G = np.exp(dxx / TEMP).sum(axis=(0, 1))             # [512]
    diag_self = np.exp(np.einsum("rid,rid->ri", xs, xs) / TEMP).sum(axis=0)
    num_xy = np.exp(dxy / TEMP).sum(axis=(0, 1))        # [512]
    return num_xy, G, diag_self


def _finish(rs_seg, num_xy, G, diag_self):
    num = num_xy + (G - diag_self) / 2.0
    den = rs_seg - num_xy - G
    loss = np.mean(-np.log(num / (den + num))) / Q
    return np.asarray(loss, dtype=np.float32)


def _numpy_fallback(x, track_idxs, y):
    """Exact general-track reference in numpy (safety net only)."""
    x64 = x.astype(np.float64)
    yf64 = y.reshape(NQ, D).astype(np.float64)
    t = track_idxs.astype(np.int64)
    y_idxs = np.tile(np.arange(N_TRACKS, dtype=np.int64), Q)
    E_xy = np.exp(x64 @ yf64.T / TEMP)
    Sx = np.zeros((N_TRACKS, NQ))
    np.add.at(Sx, t, E_xy)
    Py = (y_idxs[:, None] == np.arange(N_TRACKS)[None, :]).astype(np.float64)
    num_xy = np.einsum("ik,ki->i", Sx, Py)
    den_xy = Sx.sum(axis=1) - num_xy
    E_xx = np.exp(x64 @ x64.T / TEMP)
    Sxx = np.zeros((N_TRACKS, M))
    np.add.at(Sxx, t, E_xx)
    Px = (t[:, None] == np.arange(N_TRACKS)[None, :]).astype(np.float64)
    G_diag = np.einsum("im,mi->i", Sxx, Px)
    diag_self = np.zeros(N_TRACKS)
    np.add.at(diag_self, t, np.diagonal(E_xx))
    num_xx = (G_diag - diag_self) / 2.0
    den_xx = Sxx.sum(axis=1) - G_diag
    num = num_xy + num_xx
    den = den_xy + den_xx
    loss = np.mean(-np.log(num / (den + num))) / Q
    return np.asarray(loss, dtype=np.float32)


def kernel(x, track_idxs, y):
    x = np.asarray(x, dtype=np.float32)
    y = np.asarray(y, dtype=np.float32)
    track_idxs = np.asarray(track_idxs)

    expected_tracks = np.arange(M, dtype=np.int64) % N_TRACKS
    if (
        x.shape != (M, D)
        or y.shape != (N_TRACKS, Q, D)
        or not np.array_equal(track_idxs.astype(np.int64), expected_tracks)
    ):
        return _numpy_fallback(x, track_idxs, y)

    from concourse.bass_utils import run_bass_kernel_spmd

    yf = np.ascontiguousarray(y.reshape(NQ, D))
    xT = np.ascontiguousarray(x.T).astype(ml_dtypes.bfloat16)    # [128, 8192]
    yfT = np.ascontiguousarray(yf.T).astype(ml_dtypes.bfloat16)  # [128, 4096]
    xT_blocks = xT.reshape(128, N_BANDS, 512)
    yfT_blocks = yfT.reshape(128, Q, 512)

    in_maps = []
    for k in range(N_CORES):
        A, B = k, (N_BANDS - 1) - k
        pairs = _core_off_blocks(k)
        rhs = np.empty((128, 25, 512), dtype=ml_dtypes.bfloat16)
        for u, (_band, c) in enumerate(pairs):
            rhs[:, u] = xT_blocks[:, c]
        rhs[:, 15] = xT_blocks[:, A]
        rhs[:, 16:24] = yfT_blocks
        rhs[:, 24] = xT_blocks[:, B]

        def subtile(band, s):
            t = 4 * band + s
            return xT[:, 128 * t: 128 * (t + 1)]

        lhsT_off = np.empty((128, GROUPS, OFF_UNITS, 128), dtype=ml_dtypes.bfloat16)
        for s in range(GROUPS):
            for u, (band, _c) in enumerate(pairs):
                lhsT_off[:, s, u, :] = subtile(band, s)
        in_maps.append(
            {
                "lhsT_off": np.ascontiguousarray(lhsT_off),
                "rhs": np.ascontiguousarray(rhs),
            }
        )

    nc = _get_module()
    res = run_bass_kernel_spmd(nc, in_maps, core_ids=list(range(N_CORES)))
    _CACHED["last_res"] = res

    # Fold rowsums by row residue (128s + p) and mirror colsums by in-block
    # column position (mod 512) -- the "all-reduce" -- on the host.
    rs_seg = np.zeros(N_TRACKS, dtype=np.float64)
    for k in range(N_CORES):
        racc = np.asarray(res.results[k]["racc"], dtype=np.float64)
        cs = np.asarray(res.results[k]["cs"], dtype=np.float64)
        # racc [128, 4, 24*128], cs [128, 4, 16*128]: rowsums per (p, s)
        per_group = racc.reshape(128, GROUPS, -1).sum(axis=2)
        per_group += cs.reshape(128, GROUPS, -1).sum(axis=2)
        rs_seg += per_group.T.reshape(N_TRACKS)  # i = 128*s + p
        # colsums: every 512-col block of cs is one rhs block; positions
        # fold mod 512
        rs_seg += cs.reshape(-1, 4, 512).sum(axis=(0, 1))

    num_xy, G, diag_self = _positive_terms(
        x.astype(np.float64), yf.astype(np.float64)
    )
    return _finish(rs_seg, num_xy, G, diag_self)
